# revision 1
# baseline (speedup 1.0000x reference)
"""Trainium2 Bass kernel for nn_BipartiteGNN (2x GCNConv + mean-pool + MLP head).

Strategy (8 NeuronCores, SPMD):
  - Nodes are permuted and bin-packed into 8 cores x 49 tiles x 128 nodes
    (balanced by in-degree).  Edges live on the core owning their dst.
  - Per conv: dense matmul t = h @ W per-core (node-sharded), rows scaled by
    dinv = rsqrt(deg+1), cast fp16, AllGather -> full 50176x256 gather table
    in every core's HBM.
  - Message gather via dma_gather (int16 idx; table split at row 32768 into
    low/high passes).  Scatter-add via PE matmuls: for each 128-edge block,
    S[e, n] = w_e * (dst_rel_e == n) built by DVE is_equal-vs-iota + scale;
    psum[tile] += S^T @ G accumulates messages per 128-node tile.
  - Epilogue h' = relu(dinv*(psum + slab) + b); slab (= dinv*t) supplies the
    self-loop term.  deg from a host-padded per-node weight layout + DVE
    reduce.  Mean-pool via indicator matmuls + AllReduce, then the small MLP
    head computed feature-major (redundantly on every core).
All host-side work is index/layout preprocessing only (permutations, one-hot
placement of verbatim input values, padding); all arithmetic on tensor values
happens on device.
"""
import sys
import heapq
import numpy as np

sys.path.insert(0, "/opt/trn_rl_repo")

import concourse.bacc as bacc
import concourse.bass as bass
import concourse.tile as tile
from concourse import mybir
from concourse import bass_utils

# problem constants (hardcoded per harness contract)
N = 50000
E = 400000
G = 128
H = 256
FIN = 4
GF = 16
A = 64

NCORES = 8
NT = 49               # node tiles per core
TPC = NT * 128        # 6272 nodes per core
NPAD = NCORES * TPC   # 50176
TSPLIT = 32768        # int16 gather-index split row
CH = 32               # gather chunk, in 128-edge blocks
CH_HI = 16            # high-stream gather chunk
PROFILE_NO_CC = False  # profiling builds replace collectives with local DMAs
ABLATE = set()         # profiling: component names to skip
PSA_BUFS = 2
PSB_BUFS = 2
PST_BUFS = 2
SBP_BUFS = 12
GAT_BUFS = 2
WK_BUFS = 2

f16 = mybir.dt.float16
f32 = mybir.dt.float32
i16 = mybir.dt.int16


# ----------------------------------------------------------------------------
# host-side preprocessing (pure index/layout work)
# ----------------------------------------------------------------------------

def _greedy_pack(node_ids, keys, bin_ids):
    """Greedily pack node_ids (sorted desc by key priority) into bins of 128
    slots, minimizing the lexicographic (primary, secondary) load key.
    keys: [n, 2] per-node load contributions. Returns {node: (bin, slot)}."""
    heap = [(0.0, 0.0, b) for b in bin_ids]
    heapq.heapify(heap)
    slots_used = {b: 0 for b in bin_ids}
    out = {}
    for i, node in enumerate(node_ids):
        pend = []
        while True:
            l0, l1, b = heapq.heappop(heap)
            if slots_used[b] < 128:
                break
            pend.append((l0, l1, b))
        for p in pend:
            heapq.heappush(heap, p)
        out[node] = (b, slots_used[b])
        slots_used[b] += 1
        heapq.heappush(heap, (l0 + float(keys[i, 0]), l1 + float(keys[i, 1]), b))
    return out


def _assign_nodes(dst, src):
    """Balanced assignment of nodes to 392 (core,tile) bins of 128 slots.
    Pass 1 balances total in-degree; pass 2 rebalances within the low/high
    regions (bins < TSPLIT//128 vs the rest) on (high-deg, low-deg) jointly,
    which preserves every edge's low/high classification while minimizing the
    per-tile block counts. Returns perm (old node id -> id in [0, NPAD))."""
    deg = np.bincount(dst, minlength=N)
    nbins = NCORES * NT
    order = np.argsort(-deg, kind="stable")
    keys = np.stack([deg[order], deg[order]], 1)
    a1 = _greedy_pack(order, keys, list(range(nbins)))
    perm = np.empty(N, np.int64)
    for node, (b, s) in a1.items():
        perm[node] = b * 128 + s
    # pass 2: regional rebalance on (B=high, A=low) in-degree
    lowreg = TSPLIT // 128
    sperm = perm[src]
    hi_edge = sperm >= TSPLIT
    degA = np.bincount(dst[~hi_edge], minlength=N)
    degB = np.bincount(dst[hi_edge], minlength=N)
    perm2 = np.empty(N, np.int64)
    for region_bins, nodes in (
        (list(range(lowreg)), np.nonzero(perm[np.arange(N)] // 128 < lowreg)[0]),
        (list(range(lowreg, nbins)), np.nonzero(perm[np.arange(N)] // 128 >= lowreg)[0]),
    ):
        nb_ = degB[nodes]
        na_ = degA[nodes]
        o = np.argsort(-(nb_ * 1000 + na_), kind="stable")
        nodes_o = nodes[o]
        keys2 = np.stack([nb_[o], na_[o]], 1)
        a2 = _greedy_pack(nodes_o, keys2, region_bins)
        for node, (b, s) in a2.items():
            perm2[node] = b * 128 + s
    return perm2


def _pack_idx_stream(stream, ch_blocks):
    """Pack an int stream (len multiple of 128) into the dma_gather int16
    layout: per chunk of ch_blocks*128 idxs -> [16, n/16] cols, replicated to
    128 partitions. Returns [128, total/16] int16."""
    total = len(stream)
    cols = []
    per = ch_blocks * 128
    for st in range(0, total, per):
        chunk = stream[st:st + per]
        m = len(chunk)
        base = chunk.reshape(m // 16, 16).T.astype(np.int16)  # [16, m/16]
        cols.append(np.tile(base, (8, 1)))
    return np.concatenate(cols, axis=1)


def _prep(inputs):
    x = np.asarray(inputs["x"], np.float32)
    ei = np.asarray(inputs["edge_index"], np.int64)
    batch = np.asarray(inputs["batch"], np.int64)
    gfeat = np.asarray(inputs["global_features"], np.float32)
    ew = np.asarray(inputs["edge_weight"], np.float32)

    src, dst = ei[0], ei[1]
    perm = _assign_nodes(dst, src)

    sperm = perm[src]
    dperm = perm[dst]
    bins = dperm // 128          # global tile id 0..391
    drel = (dperm % 128).astype(np.float32)
    low = sperm < TSPLIT

    nbins = NCORES * NT
    # order edges by (bin, half)
    key = bins * 2 + (~low).astype(np.int64)
    eorder = np.argsort(key, kind="stable")
    key_s = key[eorder]
    cnt = np.bincount(key_s, minlength=nbins * 2)
    offs = np.concatenate([[0], np.cumsum(cnt)])
    cnt_lo = cnt[0::2]
    cnt_hi = cnt[1::2]
    KA = max(1, int(np.ceil(cnt_lo.max() / 128)))
    KB = max(1, int(np.ceil(cnt_hi.max() / 128))) if cnt_hi.max() > 0 else 0
    B = KA + KB

    s_s = sperm[eorder]
    d_s = drel[eorder]
    w_s = ew[eorder]

    idx_lo = np.zeros((nbins, KA * 128), np.int64)
    idx_hi = np.zeros((nbins, max(KB, 1) * 128), np.int64)
    dr_all = np.zeros((nbins, B * 128), np.float32)
    w_all = np.zeros((nbins, B * 128), np.float32)
    for b in range(nbins):
        lo0, lo1 = offs[2 * b], offs[2 * b + 1]
        hi1 = offs[2 * b + 2]
        nl = lo1 - lo0
        nh = hi1 - lo1
        idx_lo[b, :nl] = s_s[lo0:lo1]
        dr_all[b, :nl] = d_s[lo0:lo1]
        w_all[b, :nl] = w_s[lo0:lo1]
        if KB:
            idx_hi[b, :nh] = s_s[lo1:hi1] - TSPLIT
            dr_all[b, KA * 128:KA * 128 + nh] = d_s[lo1:hi1]
            w_all[b, KA * 128:KA * 128 + nh] = w_s[lo1:hi1]

    # per-node padded weight lists for deg (over ALL in-edges)
    Dmax = max(1, int(np.bincount(dperm, minlength=NPAD).max()))
    nodesort = np.argsort(dperm, kind="stable")
    dp_s = dperm[nodesort]
    w_ns = ew[nodesort]
    noffs = np.concatenate([[0], np.cumsum(np.bincount(dp_s, minlength=NPAD))])
    pos = np.arange(len(dp_s)) - noffs[dp_s]
    wpad = np.zeros(NPAD * Dmax, np.float32)
    wpad[dp_s * Dmax + pos] = w_ns
    wpad = wpad.reshape(NPAD, Dmax)

    # graph of each permuted node; dummies -> -1
    gid = np.full(NPAD, -1, np.int64)
    gid[perm] = batch

    cntg = np.bincount(batch, minlength=G).astype(np.float32)
    recip = 1.0 / np.maximum(cntg, 1.0)

    # x rows in permuted order (+ ones row), transposed
    xp = np.zeros((NPAD, FIN), np.float32)
    xp[perm] = x

    # weights
    W_emb = np.asarray(inputs["W_emb"], np.float32)
    b_emb = np.asarray(inputs["b_emb"], np.float32)
    # feature-major embed: lhsT chunks [4, 2, 128], bias as per-partition cols
    Wembc = np.stack([W_emb[:, :128], W_emb[:, 128:]], 1).astype(np.float16)
    bemb_col = np.stack([b_emb[:128], b_emb[128:]], 1).astype(np.float32)  # [128,2]

    def wchunks(Wm):  # [256,256] -> [128, 2, 256]
        Wm = np.asarray(Wm, np.float32)
        return np.stack([Wm[:128], Wm[128:]], 1).astype(np.float16)

    W1c = wchunks(inputs["W1"])
    W2c = wchunks(inputs["W2"])
    b1b = np.broadcast_to(np.asarray(inputs["b1"], np.float32)[None, :], (128, H)).astype(np.float16).copy()
    b2b = np.broadcast_to(np.asarray(inputs["b2"], np.float32)[None, :], (128, H)).astype(np.float16).copy()

    Wg = np.asarray(inputs["Wg"], np.float32)       # [16,256]
    Wg_l = np.stack([Wg[:, :128], Wg[:, 128:]], 1).astype(np.float16)   # [16,2,128]
    bg = np.asarray(inputs["bg"], np.float32)
    bg_col = np.stack([bg[:128], bg[128:]], 1)       # [128,2]
    Wh1 = np.asarray(inputs["Wh1"], np.float32)      # [512,256]
    Wh1_l = np.zeros((128, 4, 2, 128), np.float16)
    for k in range(4):
        for m in range(2):
            Wh1_l[:, k, m, :] = Wh1[k * 128:(k + 1) * 128, m * 128:(m + 1) * 128]
    bh1 = np.asarray(inputs["bh1"], np.float32)
    bh1_col = np.stack([bh1[:128], bh1[128:]], 1)    # [128,2]
    Wh2 = np.asarray(inputs["Wh2"], np.float32)      # [256,64]
    Wh2_l = np.stack([Wh2[:128], Wh2[128:]], 1).astype(np.float16)  # [128,2,64]
    bh2_col = np.asarray(inputs["bh2"], np.float32)[:, None]        # [64,1]
    gfT = gfeat.T.astype(np.float16)                 # [16,128]
    recipb = np.broadcast_to(recip[None, None, :], (128, 2, G)).astype(np.float32).copy()

    in_maps = []
    for c in range(NCORES):
        sl = slice(c * NT, (c + 1) * NT)
        nsl = slice(c * TPC, (c + 1) * TPC)
        # [49, KA*128] -> stream -> packed idx
        ilo = idx_lo[sl].reshape(-1)
        ihi = idx_hi[sl].reshape(-1)
        # dst_rel / w columns: [49, B, 128] -> [128, 49*B]
        drc = dr_all[sl].reshape(NT, B, 128).transpose(2, 0, 1).reshape(128, NT * B)
        wc = w_all[sl].reshape(NT, B, 128).transpose(2, 0, 1).reshape(128, NT * B)
        wp = wpad[nsl].reshape(NT, 128, Dmax).transpose(1, 0, 2)  # [128,49,Dmax]
        wpT = wpad[nsl].reshape(NT, 128, Dmax)                     # [49,128,Dmax]
        # pooling indicator [128, 49, G]
        gl = gid[nsl].reshape(NT, 128).T  # [128, 49]
        pind = np.zeros((128, NT, G), np.float16)
        pp, tt = np.nonzero(gl >= 0)
        pind[pp, tt, gl[pp, tt]] = 1.0
        xT5 = xp[nsl].T.astype(np.float16)  # [4, TPC]

        in_maps.append({
            "xT": xT5,
            "Wembc": Wembc, "bemb_col": bemb_col,
            "W1c": W1c, "W2c": W2c, "b1b": b1b, "b2b": b2b,
            "idx_lo": _pack_idx_stream(ilo, CH),
            "idx_hi": _pack_idx_stream(ihi, CH_HI) if KB else np.zeros((128, 8), np.int16),
            "drcols": drc.astype(np.float16),
            "wcols": wc.astype(np.float16),
            "wpad": wp.astype(np.float32).copy(),
            "pind": pind,
            "Wg_l": Wg_l, "bg_col": bg_col.astype(np.float32).copy(),
            "Wh1_l": Wh1_l, "bh1_col": bh1_col.astype(np.float32).copy(),
            "Wh2_l": Wh2_l, "bh2_col": bh2_col.astype(np.float32).copy(),
            "gfT": gfT, "recipb": recipb,
        })
    meta = dict(KA=KA, KB=KB, B=B, Dmax=Dmax)
    return in_maps, meta


# ----------------------------------------------------------------------------
# device program
# ----------------------------------------------------------------------------

def _build(meta):
    KA, KB, B, Dmax = meta["KA"], meta["KB"], meta["B"], meta["Dmax"]
    nc = bacc.Bacc("TRN2", target_bir_lowering=False, debug=False,
                   num_devices=NCORES)

    def din(name, shape, dt):
        return nc.dram_tensor(name, shape, dt, kind="ExternalInput")

    xT_d = din("xT", [FIN, TPC], f16)
    Wembc_d = din("Wembc", [FIN, 2, 128], f16)
    bemb_d = din("bemb_col", [128, 2], f32)
    W1c_d = din("W1c", [128, 2, H], f16)
    W2c_d = din("W2c", [128, 2, H], f16)
    b1b_d = din("b1b", [128, H], f16)
    b2b_d = din("b2b", [128, H], f16)
    nlowcols = ((NT * KA + CH - 1) // CH * CH) * 8  # packed cols, last chunk padded view
    idx_lo_d = din("idx_lo", [128, NT * KA * 8], i16)
    idx_hi_d = din("idx_hi", [128, (NT * KB * 8) if KB else 8], i16)
    drcols_d = din("drcols", [128, NT * B], f16)
    wcols_d = din("wcols", [128, NT * B], f16)
    wpad_d = din("wpad", [128, NT, Dmax], f32)
    pind_d = din("pind", [128, NT, G], f16)
    Wg_d = din("Wg_l", [GF, 2, 128], f16)
    bg_d = din("bg_col", [128, 2], f32)
    Wh1_d = din("Wh1_l", [128, 4, 2, 128], f16)
    bh1_d = din("bh1_col", [128, 2], f32)
    Wh2_d = din("Wh2_l", [128, 2, A], f16)
    bh2_d = din("bh2_col", [A, 1], f32)
    gfT_d = din("gfT", [GF, G], f16)
    recipb_d = din("recipb", [128, 2, G], f32)
    out_q = nc.dram_tensor("out_q", [A, G], f32, kind="ExternalOutput")

    RG = [list(range(NCORES))]

    with tile.TileContext(nc) as tc:
        with tc.tile_pool(name="const", bufs=1) as cst, \
             tc.tile_pool(name="big", bufs=1) as big, \
             tc.tile_pool(name="work", bufs=WK_BUFS) as wk, \
             tc.tile_pool(name="sbuild", bufs=SBP_BUFS) as sbp, \
             tc.tile_pool(name="gat", bufs=GAT_BUFS) as gp, \
             tc.tile_pool(name="psA", bufs=PSA_BUFS, space="PSUM") as psA, \
             tc.tile_pool(name="psB", bufs=PSB_BUFS, space="PSUM") as psB, \
             tc.tile_pool(name="psP", bufs=2, space="PSUM") as psP, \
             tc.tile_pool(name="psT", bufs=PST_BUFS, space="PSUM") as psT, \
             tc.tile_pool(name="dram", bufs=1, space="DRAM") as dr:

            # ---- constants to SBUF
            def load(pool, dram, shape, dt, tag, eng=nc.scalar):
                t = pool.tile(shape, dt, tag=tag)
                eng.dma_start(out=t[:], in_=dram.ap())
                return t

            xT = load(cst, xT_d, [FIN, TPC], f16, "xT")
            Wembc = load(cst, Wembc_d, [FIN, 2, 128], f16, "Wembc")
            bembc = load(cst, bemb_d, [128, 2], f32, "bembc")
            W1c = load(cst, W1c_d, [128, 2, H], f16, "W1c")
            W2c = load(cst, W2c_d, [128, 2, H], f16, "W2c")
            b1b = load(cst, b1b_d, [128, H], f16, "b1b")
            b2b = load(cst, b2b_d, [128, H], f16, "b2b")
            drcols = load(cst, drcols_d, [128, NT * B, 1], f16, "drcols")
            wcols = load(cst, wcols_d, [128, NT * B, 1], f16, "wcols")
            wpad = load(cst, wpad_d, [128, NT, Dmax], f32, "wpad")
            Wg_sb = load(cst, Wg_d, [GF, 2, 128], f16, "Wg")
            bg_sb = load(cst, bg_d, [128, 2], f32, "bg")
            Wh1_sb = load(cst, Wh1_d, [128, 4, 2, 128], f16, "Wh1")
            bh1_sb = load(cst, bh1_d, [128, 2], f32, "bh1")
            Wh2_sb = load(cst, Wh2_d, [128, 2, A], f16, "Wh2")
            bh2_sb = load(cst, bh2_d, [A, 1], f32, "bh2")
            gfT_sb = load(cst, gfT_d, [GF, G], f16, "gfT")
            recipb = load(cst, recipb_d, [128, 2, G], f32, "recipb")

            from concourse.masks import make_identity
            ident = cst.tile([128, 128], f16, tag="ident")
            make_identity(nc, ident[:])
            iota = cst.tile([128, 1, 128], f16, tag="iota")
            nc.gpsimd.iota(iota[:, 0, :], pattern=[[1, 128]], base=0,
                           channel_multiplier=0,
                           allow_small_or_imprecise_dtypes=True)

            # ---- deg / dinv
            deg = cst.tile([128, NT], f32, tag="deg")
            nc.vector.tensor_reduce(deg[:], wpad[:], axis=mybir.AxisListType.X,
                                    op=mybir.AluOpType.add)
            sq = cst.tile([128, NT], f32, tag="sq")
            nc.scalar.activation(sq[:], deg[:],
                                 mybir.ActivationFunctionType.Sqrt, bias=1.0)
            dinv = cst.tile([128, NT], f32, tag="dinv")
            nc.vector.reciprocal(dinv[:], sq[:])
            sq_h = cst.tile([128, NT], f16, tag="sq_h")
            nc.scalar.activation(sq_h[:], deg[:],
                                 mybir.ActivationFunctionType.Sqrt, bias=1.0)
            # row layout of sqrt(deg+1) on partition 0 for the K=1 bias matmul
            sqrow = cst.tile([1, NT * 128], f16, tag="sqrow")
            for t in range(NT):
                nc.gpsimd.dma_start(out=sqrow[0:1, t * 128:(t + 1) * 128],
                                    in_=sq_h[:, t:t + 1])

            # ---- persistent activations
            hT = big.tile([128, 2, TPC], f16, tag="hT")       # feature-major
            slab = big.tile([128, NT, H], f16, tag="slab")    # dinv*(h@W), then dinv*slab+b in-place
            h2 = big.tile([128, NT, H], f16, tag="h2")        # conv2 output

            # ---- embed, feature-major: hT[f, n] = relu(W_emb^T @ x^T + b)
            for g0 in range(0, TPC, 512):
                gn = min(512, TPC - g0)
                for k in range(2):
                    pa = psA.tile([128, 512], f32, tag="psA",
                                  name=f"psE{g0}_{k}")
                    nc.tensor.matmul(out=pa[:, :gn], lhsT=Wembc[:, k, :],
                                     rhs=xT[:, g0:g0 + gn],
                                     start=True, stop=True)
                    nc.scalar.activation(hT[:, k, g0:g0 + gn], pa[:, :gn],
                                         mybir.ActivationFunctionType.Relu,
                                         bias=bembc[:, k:k + 1])

            # ---- two conv layers
            for conv in range(2):
                Wc = W1c if conv == 0 else W2c
                bb = b1b if conv == 0 else b2b

                # dense: slab = dinv * (h @ Wc)   (t kept only via slab)
                slab_d = dr.tile([TPC, H], f16, tag=f"slab_d{conv}")
                slab_dv = slab_d[:].rearrange("(t p) f -> p t f", p=128)
                for t in range(NT):
                    pa = psA.tile([128, H], f32, tag="psA")
                    if "dense" not in ABLATE:
                        for k in range(2):
                            nc.tensor.matmul(out=pa[:],
                                             lhsT=hT[:, k, t * 128:(t + 1) * 128],
                                             rhs=Wc[:, k, :],
                                             start=(k == 0), stop=(k == 1))
                    else:
                        nc.vector.tensor_copy(pa[:], slab[:, t, :])
                    if "slabevac" not in ABLATE:
                        nc.scalar.activation(slab[:, t, :], pa[:],
                                             mybir.ActivationFunctionType.Copy,
                                             scale=dinv[:, t:t + 1])
                        nc.sync.dma_start(out=slab_dv[:, t, :], in_=slab[:, t, :])

                # AllGather -> table
                table = dr.tile([NPAD, H], f16, tag=f"table{conv}",
                                addr_space="Shared")
                if PROFILE_NO_CC:
                    nc.gpsimd.dma_start(out=table[0:TPC, :], in_=slab_d[:])
                else:
                    nc.gpsimd.collective_compute(
                        "AllGather", mybir.AluOpType.bypass, replica_groups=RG,
                        ins=[slab_d[:]], outs=[table[:]])

                # scatter: psum[tile] = sum_blocks S^T @ G  (+ self + bias)
                nlow = NT * KA
                nhigh = NT * KB
                glow_tiles = {}
                ghigh_tiles = {}

                def ensure_gather(stream_blocks, bidx, tiles_map, idx_d, tbl_view,
                                  tag, qn=0, ch=CH):
                    k = bidx // ch
                    if k in tiles_map:
                        return tiles_map[k]
                    nb = min(ch, stream_blocks - k * ch)
                    gt = gp.tile([128, ch, H], f16, tag=tag)
                    it = wk.tile([128, ch * 8], i16, tag="idxt")
                    nc.scalar.dma_start(
                        out=it[:, :nb * 8],
                        in_=idx_d.ap()[:, k * ch * 8:k * ch * 8 + nb * 8])
                    if "gather" not in ABLATE:
                        nc.gpsimd.dma_gather(
                            out_ap=gt[:, :nb, :], in_ap=tbl_view,
                            idxs_ap=it[:, :nb * 8], num_idxs=nb * 128,
                            num_idxs_reg=nb * 128, elem_size=H, queue_num=qn,
                            single_packet=False)
                    tiles_map[k] = gt
                    return gt

                for t in range(NT):
                    pb = psB.tile([128, H], f32, tag="psB")
                    S_all = sbp.tile([128, B, 128], f16, tag="S")
                    if "sbuild" not in ABLATE:
                        nc.vector.tensor_tensor(
                            S_all[:],
                            drcols[:, t * B:(t + 1) * B, :].to_broadcast([128, B, 128]),
                            iota[:].to_broadcast([128, B, 128]),
                            op=mybir.AluOpType.is_equal)
                        nc.vector.tensor_tensor(
                            S_all[:], S_all[:],
                            wcols[:, t * B:(t + 1) * B, :].to_broadcast([128, B, 128]),
                            op=mybir.AluOpType.mult)
                    nmm = KA + KB + 2
                    mm = 0
                    for j in range(KA):
                        b = t * KA + j
                        gt = ensure_gather(nlow, b, glow_tiles, idx_lo_d,
                                           table[:], "glow", qn=0)
                        if "scatter" not in ABLATE:
                            nc.tensor.matmul(out=pb[:], lhsT=S_all[:, j, :],
                                             rhs=gt[:, b % CH, :],
                                             start=(mm == 0), stop=False)
                        elif mm == 0:
                            nc.vector.tensor_copy(pb[:], slab[:, t, :])
                        mm += 1
                    for j in range(KB):
                        b = t * KB + j
                        gt = ensure_gather(nhigh, b, ghigh_tiles, idx_hi_d,
                                           table[TSPLIT:, :], "ghigh", qn=0,
                                           ch=CH_HI)
                        if "scatter" not in ABLATE:
                            nc.tensor.matmul(out=pb[:], lhsT=S_all[:, KA + j, :],
                                             rhs=gt[:, b % CH_HI, :],
                                             start=(mm == 0), stop=False)
                        elif mm == 0:
                            nc.vector.tensor_copy(pb[:], slab[:, t, :])
                        mm += 1
                    # epilogue: h' = relu(dinv*(pb + slab_t) + b)
                    # self-loop + bias into the same psum group:
                    # psum += I @ slab_t + sqrow_t^T (x) b, then
                    # relu(dinv * psum) = relu(dinv*B + dinv*slab + b)
                    nc.tensor.matmul(out=pb[:], lhsT=ident[:],
                                     rhs=slab[:, t, :],
                                     start=False, stop=False)
                    nc.tensor.matmul(out=pb[:],
                                     lhsT=sqrow[0:1, t * 128:(t + 1) * 128],
                                     rhs=bb[0:1, :],
                                     start=False, stop=True)
                    if conv == 0:
                        hsc = wk.tile([128, H], f16, tag="hscratch")
                        nc.scalar.activation(hsc[:], pb[:],
                                             mybir.ActivationFunctionType.Relu,
                                             scale=dinv[:, t:t + 1])
                        for k in range(2):
                            if "transpose" in ABLATE:
                                continue
                            pt = psT.tile([128, 128], f16, tag="psT",
                                          name=f"ptC{t}_{k}")
                            nc.tensor.transpose(
                                pt[:], hsc[:, k * 128:(k + 1) * 128], ident[:])
                            nc.scalar.activation(
                                hT[:, k, t * 128:(t + 1) * 128], pt[:],
                                mybir.ActivationFunctionType.Copy)
                    else:
                        nc.scalar.activation(h2[:, t, :], pb[:],
                                             mybir.ActivationFunctionType.Relu,
                                             scale=dinv[:, t:t + 1])

            # ---- mean pool (feature-major partial sums) + AllReduce
            pool_ps = [psP.tile([128, G], f32, tag="psP", name=f"pool_ps{i}")
                       for i in range(2)]
            for t in range(NT):
                pind_t = wk.tile([128, G], f16, tag="pind", name=f"pind{t}")
                nc.scalar.dma_start(out=pind_t[:], in_=pind_d.ap()[:, t, :])
                for m in range(2):
                    nc.tensor.matmul(out=pool_ps[m][:],
                                     lhsT=h2[:, t, m * 128:(m + 1) * 128],
                                     rhs=pind_t[:],
                                     start=(t == 0), stop=(t == NT - 1))
            poolT = wk.tile([128, 2, G], f32, tag="poolT")
            for m in range(2):
                nc.vector.tensor_copy(poolT[:, m, :], pool_ps[m][:])
            pool_in = dr.tile([128, 2 * G], f32, tag="pool_in")
            pool_out = dr.tile([128, 2 * G], f32, tag="pool_out",
                               addr_space="Shared")
            nc.gpsimd.dma_start(out=pool_in[:],
                                in_=poolT[:].rearrange("p m g -> p (m g)"))
            if PROFILE_NO_CC:
                nc.gpsimd.dma_start(out=pool_out[:], in_=pool_in[:])
            else:
                nc.gpsimd.collective_compute(
                    "AllReduce", mybir.AluOpType.add, replica_groups=RG,
                    ins=[pool_in[:]], outs=[pool_out[:]])
            poolAR = wk.tile([128, 2, G], f32, tag="poolAR")
            nc.gpsimd.dma_start(
                out=poolAR[:],
                in_=pool_out[:].rearrange("p (m g) -> p m g", m=2))

            combT = wk.tile([128, 4, G], f16, tag="combT")
            nc.vector.tensor_tensor(combT[:, 0:2, :], poolAR[:], recipb[:],
                                    op=mybir.AluOpType.mult)
            # global embed: relu(Wg^T @ gfT + bg)
            for m in range(2):
                pe = psP.tile([128, G], f32, tag="psP")
                nc.tensor.matmul(out=pe[:], lhsT=Wg_sb[:, m, :], rhs=gfT_sb[:],
                                 start=True, stop=True)
                nc.scalar.activation(combT[:, 2 + m, :], pe[:],
                                     mybir.ActivationFunctionType.Relu,
                                     bias=bg_sb[:, m:m + 1])
            # q1 = relu(Wh1^T @ comb + bh1)
            q1T = wk.tile([128, 2, G], f16, tag="q1T")
            for m in range(2):
                pq = psP.tile([128, G], f32, tag="psP")
                for k in range(4):
                    nc.tensor.matmul(out=pq[:], lhsT=Wh1_sb[:, k, m, :],
                                     rhs=combT[:, k, :],
                                     start=(k == 0), stop=(k == 3))
                nc.scalar.activation(q1T[:, m, :], pq[:],
                                     mybir.ActivationFunctionType.Relu,
                                     bias=bh1_sb[:, m:m + 1])
            # q = Wh2^T @ q1 + bh2
            pqf = psP.tile([A, G], f32, tag="psP")
            for k in range(2):
                nc.tensor.matmul(out=pqf[:], lhsT=Wh2_sb[:, k, :],
                                 rhs=q1T[:, k, :], start=(k == 0), stop=(k == 1))
            qT = wk.tile([A, G], f32, tag="qT")
            nc.scalar.activation(qT[:], pqf[:],
                                 mybir.ActivationFunctionType.Identity,
                                 bias=bh2_sb[:])
            nc.sync.dma_start(out=out_q.ap(), in_=qT[:])

    nc.compile()
    return nc


_CACHE = {}


def kernel(**inputs):
    in_maps, meta = _prep(inputs)
    key = (meta["KA"], meta["KB"], meta["Dmax"])
    if key not in _CACHE:
        _CACHE[key] = _build(meta)
    nc = _CACHE[key]
    res = bass_utils.run_bass_kernel_spmd(
        nc, in_maps, core_ids=list(range(NCORES)), trace=False)
    q = res.results[0]["out_q"].T.astype(np.float32).copy()
    # stash for test harness reuse (timing)
    kernel._last = (nc, in_maps)
    return q

